# revision 17
# baseline (speedup 1.0000x reference)
"""Trainium2 Bass kernel for nn_AttentionControl (recurrent attention glimpse
network step, eval mode).

Contract: kernel(**inputs) takes the FULL unsharded inputs (B=512) and
returns the full [512, 256] output. Pure data parallel across 8 NeuronCores
(64 samples each). All compute runs on device; the host only reshapes/pads
inputs (data-independent layout prep) and folds the constant linear ops
(crop-select + bilinear antialias resize) into the dense weights.

v2 design (per core, per body) — minimal instruction count, no PE transposes:
  1. loc both ways via flipped matmuls on const X^T chunks:
     sample-major ploc [64,2] (for the exact fp32 index chain) and
     unit-major plocT [2,64] (feeds the hl matmul directly).
  2. fp32 index chain on DVE: clip, lr=RNE(112*loc+112) via +2^23,
     band=floor(l1/32) via +0.51/RNE, A = sample_base + band*(BANDE-32)
     + l0*96 + l1 (all exact in f32, < 2^24).
  3. ONE indirect element-granularity gather per half: partition p = sample,
     reads 32 consecutive 96-wide band rows (whole half-window incl. 33%
     junk cols) as one contiguous bf16 run. The image is stored banded
     (8 col-bands of width 96 at stride 32) in bf16.
  4. ONE xbar DMA-transpose per half: [64, 3072] -> [128, 24, 64] bf16 =
     pixel-major chunks. Junk columns are handled by ZERO rows folded into
     the weights (wwin expanded 4096 -> 6144 gathered positions).
  5. 48 accumulating bf16 matmuls (lhsT = folded weights [128,128],
     rhs = xbar chunks [128,64]) -> phg [HG, 64] in PSUM.
  6. hgT/hlT via scalar-engine Relu+bias (per-partition bias), then g
     sample-major via flipped matmuls (lhsT = hgT/hlT [128,64], rhs =
     W_gs/W_ls [128,256]) + K=1 ones-matmul adding the g bias; Relu on DVE;
     direct [64,256] f32 DMA out. No transposes anywhere in the tail.
"""
import numpy as np
import ml_dtypes

# ---------------- problem constants (hardcoded per contract) ----------------
B = 512
STATE = 512
S = 224
G = 256
HG = 128
HL = 128
TSB = 768
N_CORES = 8
NS = B // N_CORES            # samples per core = 64
PAD = 32                     # window pad (window = 64x64 around loc)
PADC_R = 64                  # right col pad so the last band (start 224) fits
PR = S + 2 * PAD             # padded rows = 288
BW = 96                      # band width
BSTRIDE = 32                 # band column stride
NBAND = 8                    # bands at column starts 0, 32, ..., 224
BANDE = PR * BW              # elements per band = 27648
SAMPE = NBAND * BANDE        # elements per sample = 221184
NH = 2                       # window halves (rows 0-31 / 32-63)
HROWS = 32                   # band rows per half
HRUN = HROWS * BW            # gathered elements per half = 3072
NCH = HRUN // 128            # xbar chunks per half = 24
IMG2_ELEMS = NS * SAMPE + 2048   # + tail pad
IMG_ROWS = IMG2_ELEMS // 1024    # img declared [IMG_ROWS, 1024] bf16 so the
                                 # src AP's innermost run is wide (desc sizing)

_F32 = np.float32
_BF16 = ml_dtypes.bfloat16


def _resize_weight_mat(d, n=16):
    """jax.image.resize 'bilinear' (triangle kernel, antialias=True) weight
    matrix [d, n]; resized = w.T @ x @ w for a [d, d] input."""
    scale = _F32(n / d)
    inv_scale = _F32(1.0) / scale
    kernel_scale = np.maximum(inv_scale, _F32(1.0))
    sample_f = (np.arange(n, dtype=_F32) + _F32(0.5)) * inv_scale - _F32(0.5)
    x = np.abs(sample_f[None, :] - np.arange(d, dtype=_F32)[:, None]) / kernel_scale
    w = np.maximum(_F32(0), _F32(1) - np.abs(x)).astype(_F32)
    total = w.sum(axis=0, keepdims=True, dtype=_F32)
    w = np.where(np.abs(total) > 1000.0 * np.finfo(_F32).eps,
                 (w / np.where(total != 0, total, 1)).astype(_F32), 0.0).astype(_F32)
    keep = (sample_f >= -0.5) & (sample_f <= d - 0.5)
    return np.where(keep[None, :], w, 0.0).astype(_F32)


def _build_wwin(W_hg):
    """Fold crop-select + resize + W_hg into Wwin [4096, 128] acting on the
    flattened 64x64 window."""
    W = np.asarray(W_hg, dtype=np.float64)
    L = np.zeros((4096, TSB), dtype=np.float64)
    for i in range(16):
        for j in range(16):
            L[(24 + i) * 64 + (24 + j), i * 16 + j] = 1.0
    w32 = _resize_weight_mat(32).astype(np.float64)
    blk32 = np.einsum("ri,cj->rcij", w32, w32).reshape(32, 32, 256)
    for r in range(32):
        for c in range(32):
            L[(16 + r) * 64 + (16 + c), 256:512] = blk32[r, c]
    w64 = _resize_weight_mat(64).astype(np.float64)
    blk64 = np.einsum("ri,cj->rcij", w64, w64).reshape(64, 64, 256)
    for r in range(64):
        for c in range(64):
            L[r * 64 + c, 512:768] = blk64[r, c]
    return (L @ W).astype(_F32)  # [4096, 128]


def _build_wwin_gathered(W_hg):
    """Expand Wwin to the gathered-position basis [2*HRUN, HG]: gathered
    element q of half h = band row (q//96), col (q%96); window pixel
    (32h + q//96, q%96) when q%96 < 64, else junk -> zero weight.
    Returns bf16 [128, 2*NCH, HG]: [pos-in-chunk, chunk, hg]."""
    wwin = _build_wwin(W_hg)                       # [4096, 128]
    wg = np.zeros((NH * HRUN, HG), np.float64)
    w4 = wwin.reshape(64, 64, HG)
    for h in range(NH):
        for r in range(HROWS):
            wg[h * HRUN + r * BW: h * HRUN + r * BW + 64] = w4[HROWS * h + r]
    wg = wg.reshape(NH * NCH, 128, HG).transpose(1, 0, 2)  # [128, 48, HG]
    return np.ascontiguousarray(wg.astype(_BF16))


# ---------------------------------------------------------------------------
# Bass program (built once, cached)
# ---------------------------------------------------------------------------
_CACHE = {}

BIG = float(2.0 ** 23)


def _build_nc(debug=False, loop_n=1, hw_loop=0, unroll=1,
              do_front=True, do_gather=True, do_xbar=True, do_win=True,
              do_tail=True, stag=False):
    from contextlib import ExitStack, nullcontext
    import concourse.bass as bass
    import concourse.mybir as mybir
    import concourse.tile as tile
    from concourse import bacc

    dt = mybir.dt
    nc = bacc.Bacc("TRN2", target_bir_lowering=False, debug=False,
                   num_devices=N_CORES)

    F32 = dt.float32
    BF16 = dt.bfloat16
    Relu = mybir.ActivationFunctionType.Relu
    Alu = mybir.AluOpType

    # ---- DRAM I/O ----
    img = nc.dram_tensor("img", [IMG_ROWS, 1024], BF16, kind="ExternalInput")
    xt_d = nc.dram_tensor("xt", [128, 4, NS], F32, kind="ExternalInput")
    wloc_d = nc.dram_tensor("wloc", [128, 4, 2], F32, kind="ExternalInput")
    wwin_d = nc.dram_tensor("wwin", [128, NH * NCH, HG], BF16, kind="ExternalInput")
    wgl_d = nc.dram_tensor("wgl", [128, 2 * G], BF16, kind="ExternalInput")  # wgs|wls
    whl_d = nc.dram_tensor("whl", [2, HL], BF16, kind="ExternalInput")
    csm_d = nc.dram_tensor("csm", [NS, 5], F32, kind="ExternalInput")   # bloc[2], sampb, ones, pad
    ccol_d = nc.dram_tensor("ccol", [128, 4], F32, kind="ExternalInput")  # bhg, bhl, bloc2 cols 0/1 (rows 0-1)
    bg_d = nc.dram_tensor("bg", [1, G + NS], F32, kind="ExternalInput")  # bias_g | ones
    out_d = nc.dram_tensor("out", [NS, G], F32, kind="ExternalOutput")
    if debug:
        dbg_loc = nc.dram_tensor("dbg_loc", [NS, 2], F32, kind="ExternalOutput")
        dbg_idx = nc.dram_tensor("dbg_idx", [NS, 1], dt.int32, kind="ExternalOutput")
        dbg_g = nc.dram_tensor("dbg_g", [NS, NH * HRUN], F32, kind="ExternalOutput")
        dbg_hg = nc.dram_tensor("dbg_hg", [HG, NS], F32, kind="ExternalOutput")

    def indirect_gather_elem(out_ap, idx_ap):
        # per-partition contiguous element-granularity gather from img flat;
        # probed HW semantics: offsets [P,1] int32, dest [P,F], each partition
        # reads F contiguous elements from flat[idx[p]].
        eng = nc.gpsimd
        out_l = eng.lower_ap_dma(out_ap, for_indirect_dma=True)
        in_l = eng.lower_ap_dma(img.ap(), for_indirect_dma=True)
        off_l = eng.lower_ap_dma(idx_ap)
        assert len(out_l) == 1 and len(in_l) == 1 and len(off_l) == 1
        in_l[0].dynamic_ap_info = mybir.DynamicAccessPatternInfo(
            c=0,
            actual_ap=out_ap.ap,
            indirect_dim_max_index=IMG2_ELEMS,
            offset_expr=[
                mybir.DynamicAccessPatternOffsetExpr(
                    coef=1,
                    aff_expr=mybir.DynamicAccessPatternOffsetExprAffExpr(
                        kind="IndirectArgId", arg_id=1),
                )
            ],
        )
        in_l.append(off_l[0])
        return eng.add_instruction(
            mybir.InstDMACopy(
                name=nc.get_next_instruction_name(),
                queue="qPoolDynamic",
                mode="Copy",
                ins=in_l,
                outs=out_l,
                oob_is_err=True,
                cce_op=mybir.AluOpType.bypass,
            ))

    with tile.TileContext(nc) as tc, ExitStack() as ctx:
        const = ctx.enter_context(tc.tile_pool(name="const", bufs=1))
        work = ctx.enter_context(tc.tile_pool(name="work", bufs=2))
        small = ctx.enter_context(tc.tile_pool(name="small", bufs=2))
        ps_f = ctx.enter_context(tc.tile_pool(name="ps_f", bufs=1, space="PSUM"))
        ps_hl = ctx.enter_context(tc.tile_pool(name="ps_hl", bufs=1, space="PSUM"))
        ps_hg = ctx.enter_context(tc.tile_pool(name="ps_hg", bufs=2, space="PSUM"))
        ps_g = ctx.enter_context(tc.tile_pool(name="ps_g", bufs=2, space="PSUM"))

        # ---- constants ----
        xt_sb = const.tile([128, 4, NS], F32, tag="xt")
        nc.sync.dma_start(xt_sb[:], xt_d.ap())
        wloc_sb = const.tile([128, 4, 2], F32, tag="wloc")
        nc.sync.dma_start(wloc_sb[:], wloc_d.ap())
        wwin_sb = const.tile([128, NH * NCH, HG], BF16, tag="wwin")
        for gi in range(4):
            sl = slice(gi * 12, gi * 12 + 12)
            nc.scalar.dma_start(wwin_sb[:, sl, :], wwin_d.ap()[:, sl, :])
        wgl_sb = const.tile([128, 2 * G], BF16, tag="wgl")
        nc.sync.dma_start(wgl_sb[:], wgl_d.ap())
        wgs_sb = wgl_sb[:, 0:G]
        wls_sb = wgl_sb[:, G:2 * G]
        whl_sb = const.tile([2, HL], BF16, tag="whl")
        nc.sync.dma_start(whl_sb[:], whl_d.ap())
        csm = const.tile([NS, 5], F32, tag="csm")
        nc.sync.dma_start(csm[:], csm_d.ap())
        bloc_sb = csm[:, 0:2]
        sampb_sb = csm[:, 2:3]
        ccol = const.tile([128, 4], F32, tag="ccol")
        nc.sync.dma_start(ccol[:], ccol_d.ap())
        bhg_sb = ccol[:, 0:1]
        bhl_sb = ccol[:, 1:2]
        bloc2_sb = ccol[0:2, 2:3]
        bg_sb = const.tile([1, G + NS], F32, tag="bg")
        nc.sync.dma_start(bg_sb[:], bg_d.ap())
        ones_sb = bg_sb[0:1, G:G + NS]

        loop_cm = (tc.For_i(0, hw_loop, 1, staggered_reset=stag)
                   if hw_loop else nullcontext())

        def stage_front(st):
            # ---- loc sample-major (fp32 index chain) ----
            ploc = ps_f.tile([NS, 2], F32, tag="ploc")
            for k in range(4):
                nc.tensor.matmul(ploc[:], xt_sb[:, k, :], wloc_sb[:, k, :],
                                 start=(k == 0), stop=(k == 3))
            loc_sb = small.tile([NS, 2], F32, tag="loc")
            nc.vector.tensor_tensor(loc_sb[:], ploc[:], bloc_sb, op=Alu.add)
            nc.vector.tensor_scalar(loc_sb[:], loc_sb[:], 1.0, -1.0,
                                    op0=Alu.min, op1=Alu.max)
            lr_sb = small.tile([NS, 2], F32, tag="lr")
            nc.vector.tensor_scalar(lr_sb[:], loc_sb[:], 112.0, 112.0,
                                    op0=Alu.mult, op1=Alu.add)
            nc.vector.tensor_scalar(lr_sb[:], lr_sb[:], BIG, BIG,
                                    op0=Alu.add, op1=Alu.subtract)
            band_sb = small.tile([NS, 1], F32, tag="band")
            nc.vector.tensor_scalar(band_sb[:], lr_sb[:, 1:2], 1.0 / BSTRIDE, 0.51,
                                    op0=Alu.mult, op1=Alu.add)
            nc.vector.tensor_scalar(band_sb[:], band_sb[:], BIG, BIG,
                                    op0=Alu.add, op1=Alu.subtract)
            # A = sampb' + (band+1)*(BANDE-32) + l0*96 + l1
            a_sb = small.tile([NS, NH], F32, tag="abase")
            t_sb = small.tile([NS, 1], F32, tag="tmp")
            nc.vector.tensor_scalar(t_sb[:], band_sb[:], float(BANDE - BSTRIDE),
                                    None, op0=Alu.mult)
            nc.vector.tensor_tensor(t_sb[:], t_sb[:], sampb_sb, op=Alu.add)
            nc.vector.tensor_scalar(a_sb[:, 0:1], lr_sb[:, 0:1], float(BW), None,
                                    op0=Alu.mult)
            nc.vector.tensor_tensor(a_sb[:, 0:1], a_sb[:, 0:1], t_sb[:], op=Alu.add)
            nc.vector.tensor_tensor(a_sb[:, 0:1], a_sb[:, 0:1], lr_sb[:, 1:2],
                                    op=Alu.add)
            idx_sb = small.tile([NS, 1], dt.int32, tag="idx")
            nc.vector.tensor_copy(idx_sb[:], a_sb[:, 0:1])
            st["idx"] = idx_sb
            st["loc"] = loc_sb

            # ---- loc unit-major -> hl ----
            plocT = ps_f.tile([2, NS], F32, tag="plocT")
            for k in range(4):
                nc.tensor.matmul(plocT[:], wloc_sb[:, k, :], xt_sb[:, k, :],
                                 start=(k == 0), stop=(k == 3))
            locT_sb = small.tile([2, NS], BF16, tag="locT")
            nc.scalar.activation(locT_sb[:], plocT[:],
                                 mybir.ActivationFunctionType.Identity,
                                 bias=bloc2_sb)
            nc.vector.tensor_scalar(locT_sb[:], locT_sb[:], 1.0, -1.0,
                                    op0=Alu.min, op1=Alu.max)
            phl = ps_hl.tile([HL, NS], F32, tag="phl")
            nc.tensor.matmul(phl[:], whl_sb[:], locT_sb[:], start=True, stop=True)
            hlT_sb = small.tile([HL, NS], BF16, tag="hlT")
            nc.scalar.activation(hlT_sb[:], phl[:], Relu, bias=bhl_sb)
            st["hlT"] = hlT_sb

        def stage_gx(st):
            # ---- one gather + one xbar transpose (whole 64x96 window) ----
            gbuf = work.tile([NS, NH * HRUN], BF16, tag="gbuf")
            st["gbuf"] = gbuf
            if do_gather:
                indirect_gather_elem(gbuf[:], st["idx"][:])
            rhsT = work.tile([128, NH * NCH, NS], BF16, tag="rhsT")
            if do_xbar:
                nc.sync.dma_start(rhsT[:], gbuf[:], transpose=True)
            st["rhsT"] = rhsT

        def stage_tail(st):
            # ---- window matmuls -> hg ----
            phg = ps_hg.tile([HG, NS], F32, tag="phg")
            if do_win:
                for c in range(NH * NCH):
                    nc.tensor.matmul(phg[:], wwin_sb[:, c, :],
                                     st["rhsT"][:, c, :],
                                     start=(c == 0),
                                     stop=(c == NH * NCH - 1))
            if not (do_win and do_tail):
                return
            hgT_sb = work.tile([HG, NS], BF16, tag="hgT")
            nc.scalar.activation(hgT_sb[:], phg[:], Relu, bias=bhg_sb)

            # ---- g = relu(W_gs^T hg + W_ls^T hl + bias), sample-major ----
            pg = ps_g.tile([NS, G], F32, tag="pg")
            nc.tensor.matmul(pg[:], hgT_sb[:], wgs_sb, start=True, stop=False)
            nc.tensor.matmul(pg[:], st["hlT"][:], wls_sb, start=False, stop=False)
            nc.tensor.matmul(pg[:], ones_sb, bg_sb[0:1, 0:G], start=False, stop=True)
            g_sb = work.tile([NS, G], F32, tag="g")
            nc.scalar.activation(g_sb[:], pg[:], Relu)
            nc.scalar.dma_start(out_d.ap(), g_sb[:])

            if debug:
                nc.sync.dma_start(dbg_loc.ap(), st["loc"][:])
                nc.sync.dma_start(dbg_idx.ap(), st["idx"][:])
                gf = work.tile([NS, NH * HRUN], F32, tag="gf")
                nc.vector.tensor_copy(gf[:], st["gbuf"][:])
                nc.sync.dma_start(dbg_g.ap(), gf[:])
                hgf = work.tile([HG, NS], F32, tag="hgf")
                nc.vector.tensor_copy(hgf[:], hgT_sb[:])
                nc.sync.dma_start(dbg_hg.ap(), hgf[:])

        # Stage-interleaved emission (1-deep software pipeline): engines
        # execute their queues in emission order, so copy k+1's cheap front
        # must be enqueued BEFORE copy k's tail to avoid head-of-line
        # blocking behind the gather/xbar latency chain.
        with loop_cm:
            ncopies = loop_n * unroll
            states = [dict() for _ in range(ncopies)]
            prev = None
            for it in range(ncopies):
                if do_front:
                    stage_front(states[it])
                    stage_gx(states[it])
                if prev is not None:
                    stage_tail(prev)
                prev = states[it]
            if prev is not None and do_front:
                stage_tail(prev)

    nc.compile()
    return nc


def _host_prep(inputs):
    """Build the per-core in_maps (pure layout transforms of the inputs)."""
    X = np.ascontiguousarray(np.asarray(inputs["output"], dtype=_F32))
    img = np.asarray(inputs["inputs"], dtype=_F32)[..., 0]
    W_loc = np.asarray(inputs["W_loc"], dtype=_F32)
    b_loc = np.asarray(inputs["b_loc"], dtype=_F32)
    W_hl = np.asarray(inputs["W_hl"], dtype=_F32)
    b_hl = np.asarray(inputs["b_hl"], dtype=_F32)
    W_gs = np.asarray(inputs["W_gs"], dtype=_F32)
    b_gs = np.asarray(inputs["b_gs"], dtype=_F32)
    W_ls = np.asarray(inputs["W_ls"], dtype=_F32)
    b_ls = np.asarray(inputs["b_ls"], dtype=_F32)
    b_hg = np.asarray(inputs["b_hg"], dtype=_F32)

    wwin_g = _build_wwin_gathered(inputs["W_hg"])     # bf16 [128, 48, HG]

    # padded [B, 288, 320] bf16 -> bands [B, 8, 288, 96]
    padded = np.pad(img, ((0, 0), (PAD, PAD), (PAD, PADC_R))).astype(_BF16)
    bands = np.stack([padded[:, :, BSTRIDE * k:BSTRIDE * k + BW]
                      for k in range(NBAND)], axis=1)

    # xt[p, k, s] = X[s, 128k + p]
    xt = np.ascontiguousarray(
        X.reshape(NS * N_CORES, 4, 128).transpose(2, 1, 0))  # [128, 4, B]
    wloc = np.ascontiguousarray(
        W_loc.reshape(4, 128, 2).transpose(1, 0, 2))          # [128, 4, 2]
    wgl = np.concatenate([W_gs, W_ls], axis=1).astype(_BF16)  # [128, 512]
    whl = W_hl.astype(_BF16)                                  # [2, 128]

    s = np.arange(NS, dtype=np.float64)
    csm = np.zeros((NS, 5), _F32)
    csm[:, 0:2] = b_loc[None, :]
    csm[:, 2] = (s * SAMPE - (BANDE - BSTRIDE)).astype(_F32)
    csm[:, 3] = 1.0
    ccol = np.zeros((128, 4), _F32)
    ccol[:, 0] = b_hg
    ccol[:, 1] = b_hl
    ccol[0:2, 2] = b_loc
    bg = np.zeros((1, G + NS), _F32)
    bg[0, 0:G] = b_gs + b_ls
    bg[0, G:] = 1.0

    in_maps = []
    for c in range(N_CORES):
        sl = slice(c * NS, (c + 1) * NS)
        imgc = np.concatenate(
            [bands[sl].reshape(-1),
             np.zeros(IMG_ROWS * 1024 - NS * SAMPE, _BF16)]).reshape(IMG_ROWS, 1024)
        in_maps.append({
            "img": imgc,
            "xt": np.ascontiguousarray(xt[:, :, sl]),
            "wloc": wloc,
            "wwin": wwin_g,
            "wgl": wgl,
            "whl": whl,
            "csm": csm,
            "ccol": ccol,
            "bg": bg,
        })
    return in_maps


def kernel(**inputs) -> np.ndarray:
    from concourse.bass_utils import run_bass_kernel_spmd

    if "nc" not in _CACHE:
        _CACHE["nc"] = _build_nc()
    nc = _CACHE["nc"]
    in_maps = _host_prep(inputs)
    res = run_bass_kernel_spmd(nc, in_maps, core_ids=list(range(N_CORES)))
    out = np.concatenate([res.results[c]["out"] for c in range(N_CORES)], axis=0)
    return out.astype(np.float32)


# revision 19
# speedup vs baseline: 1.0127x; 1.0127x over previous
"""Trainium2 Bass kernel for nn_AttentionControl (recurrent attention glimpse
network step, eval mode).

Contract: kernel(**inputs) takes the FULL unsharded inputs (B=512) and
returns the full [512, 256] output. Pure data parallel across 8 NeuronCores
(64 samples each). All compute runs on device; the host only reshapes/pads
inputs (data-independent layout prep) and folds the constant linear ops
(crop-select + bilinear antialias resize) into the dense weights.

v2 design (per core, per body) — minimal instruction count, no PE transposes:
  1. loc both ways via flipped matmuls on const X^T chunks:
     sample-major ploc [64,2] (for the exact fp32 index chain) and
     unit-major plocT [2,64] (feeds the hl matmul directly).
  2. fp32 index chain on DVE: clip, lr=RNE(112*loc+112) via +2^23,
     band=floor(l1/32) via +0.51/RNE, A = sample_base + band*(BANDE-32)
     + l0*96 + l1 (all exact in f32, < 2^24).
  3. ONE indirect element-granularity gather per half: partition p = sample,
     reads 32 consecutive 96-wide band rows (whole half-window incl. 33%
     junk cols) as one contiguous bf16 run. The image is stored banded
     (8 col-bands of width 96 at stride 32) in bf16.
  4. ONE xbar DMA-transpose per half: [64, 3072] -> [128, 24, 64] bf16 =
     pixel-major chunks. Junk columns are handled by ZERO rows folded into
     the weights (wwin expanded 4096 -> 6144 gathered positions).
  5. 48 accumulating bf16 matmuls (lhsT = folded weights [128,128],
     rhs = xbar chunks [128,64]) -> phg [HG, 64] in PSUM.
  6. hgT/hlT via scalar-engine Relu+bias (per-partition bias), then g
     sample-major via flipped matmuls (lhsT = hgT/hlT [128,64], rhs =
     W_gs/W_ls [128,256]) + K=1 ones-matmul adding the g bias; Relu on DVE;
     direct [64,256] f32 DMA out. No transposes anywhere in the tail.
"""
import numpy as np
import ml_dtypes

# ---------------- problem constants (hardcoded per contract) ----------------
B = 512
STATE = 512
S = 224
G = 256
HG = 128
HL = 128
TSB = 768
N_CORES = 8
NS = B // N_CORES            # samples per core = 64
PAD = 32                     # window pad (window = 64x64 around loc)
PADC_R = 64                  # right col pad so the last band (start 224) fits
PR = S + 2 * PAD             # padded rows = 288
BW = 96                      # band width
BSTRIDE = 32                 # band column stride
NBAND = 8                    # bands at column starts 0, 32, ..., 224
BANDE = PR * BW              # elements per band = 27648
SAMPE = NBAND * BANDE        # elements per sample = 221184
NH = 2                       # window halves (rows 0-31 / 32-63)
HROWS = 32                   # band rows per half
HRUN = HROWS * BW            # gathered elements per half = 3072
NCH = HRUN // 128            # xbar chunks per half = 24
IMG2_ELEMS = NS * SAMPE + 2048   # + tail pad
IMG_ROWS = IMG2_ELEMS // 1024    # img declared [IMG_ROWS, 1024] bf16 so the
                                 # src AP's innermost run is wide (desc sizing)

_F32 = np.float32
_BF16 = ml_dtypes.bfloat16


def _resize_weight_mat(d, n=16):
    """jax.image.resize 'bilinear' (triangle kernel, antialias=True) weight
    matrix [d, n]; resized = w.T @ x @ w for a [d, d] input."""
    scale = _F32(n / d)
    inv_scale = _F32(1.0) / scale
    kernel_scale = np.maximum(inv_scale, _F32(1.0))
    sample_f = (np.arange(n, dtype=_F32) + _F32(0.5)) * inv_scale - _F32(0.5)
    x = np.abs(sample_f[None, :] - np.arange(d, dtype=_F32)[:, None]) / kernel_scale
    w = np.maximum(_F32(0), _F32(1) - np.abs(x)).astype(_F32)
    total = w.sum(axis=0, keepdims=True, dtype=_F32)
    w = np.where(np.abs(total) > 1000.0 * np.finfo(_F32).eps,
                 (w / np.where(total != 0, total, 1)).astype(_F32), 0.0).astype(_F32)
    keep = (sample_f >= -0.5) & (sample_f <= d - 0.5)
    return np.where(keep[None, :], w, 0.0).astype(_F32)


def _build_wwin(W_hg):
    """Fold crop-select + resize + W_hg into Wwin [4096, 128] acting on the
    flattened 64x64 window."""
    W = np.asarray(W_hg, dtype=np.float64)
    L = np.zeros((4096, TSB), dtype=np.float64)
    for i in range(16):
        for j in range(16):
            L[(24 + i) * 64 + (24 + j), i * 16 + j] = 1.0
    w32 = _resize_weight_mat(32).astype(np.float64)
    blk32 = np.einsum("ri,cj->rcij", w32, w32).reshape(32, 32, 256)
    for r in range(32):
        for c in range(32):
            L[(16 + r) * 64 + (16 + c), 256:512] = blk32[r, c]
    w64 = _resize_weight_mat(64).astype(np.float64)
    blk64 = np.einsum("ri,cj->rcij", w64, w64).reshape(64, 64, 256)
    for r in range(64):
        for c in range(64):
            L[r * 64 + c, 512:768] = blk64[r, c]
    return (L @ W).astype(_F32)  # [4096, 128]


def _build_wwin_chunks(W_hg):
    """Wwin [4096, HG] chunked for the compact window: [128, 32, HG] =
    [pos-in-chunk, chunk, hg]."""
    wwin = _build_wwin(W_hg)                       # [4096, 128]
    wg = wwin.reshape(32, 128, HG).transpose(1, 0, 2)
    return np.ascontiguousarray(wg.astype(_BF16))


# ---------------------------------------------------------------------------
# Bass program (built once, cached)
# ---------------------------------------------------------------------------
_CACHE = {}

BIG = float(2.0 ** 23)


def _build_nc(debug=False, loop_n=1, hw_loop=0, unroll=1,
              do_front=True, do_gather=True, do_xbar=True, do_win=True,
              do_tail=True, stag=False):
    from contextlib import ExitStack, nullcontext
    import concourse.bass as bass
    import concourse.mybir as mybir
    import concourse.tile as tile
    from concourse import bacc

    dt = mybir.dt
    nc = bacc.Bacc("TRN2", target_bir_lowering=False, debug=False,
                   num_devices=N_CORES)

    F32 = dt.float32
    BF16 = dt.bfloat16
    Relu = mybir.ActivationFunctionType.Relu
    Alu = mybir.AluOpType

    # ---- DRAM I/O ----
    img = nc.dram_tensor("img", [IMG_ROWS, 1024], BF16, kind="ExternalInput")
    xt_d = nc.dram_tensor("xt", [128, 4, NS], F32, kind="ExternalInput")
    wloc_d = nc.dram_tensor("wloc", [128, 4, 2], F32, kind="ExternalInput")
    wwin_d = nc.dram_tensor("wwin", [128, 32, HG], BF16, kind="ExternalInput")
    wgl_d = nc.dram_tensor("wgl", [128, 2 * G], BF16, kind="ExternalInput")  # wgs|wls
    whl_d = nc.dram_tensor("whl", [2, HL], BF16, kind="ExternalInput")
    csm_d = nc.dram_tensor("csm", [NS, 5], F32, kind="ExternalInput")   # bloc[2], sampb, ones, pad
    ccol_d = nc.dram_tensor("ccol", [128, 4], F32, kind="ExternalInput")  # bhg, bhl, bloc2 cols 0/1 (rows 0-1)
    bg_d = nc.dram_tensor("bg", [1, G + NS], F32, kind="ExternalInput")  # bias_g | ones
    out_d = nc.dram_tensor("out", [NS, G], F32, kind="ExternalOutput")
    if debug:
        dbg_loc = nc.dram_tensor("dbg_loc", [NS, 2], F32, kind="ExternalOutput")
        dbg_idx = nc.dram_tensor("dbg_idx", [NS, 1], dt.int32, kind="ExternalOutput")
        dbg_g = nc.dram_tensor("dbg_g", [NS, NH * HRUN], F32, kind="ExternalOutput")
        dbg_hg = nc.dram_tensor("dbg_hg", [HG, NS], F32, kind="ExternalOutput")

    def indirect_gather_elem(out_ap, idx_ap):
        # per-partition contiguous element-granularity gather from img flat;
        # probed HW semantics: offsets [P,1] int32, dest [P,F], each partition
        # reads F contiguous elements from flat[idx[p]].
        eng = nc.gpsimd
        out_l = eng.lower_ap_dma(out_ap, for_indirect_dma=True)
        in_l = eng.lower_ap_dma(img.ap(), for_indirect_dma=True)
        off_l = eng.lower_ap_dma(idx_ap)
        assert len(out_l) == 1 and len(in_l) == 1 and len(off_l) == 1
        in_l[0].dynamic_ap_info = mybir.DynamicAccessPatternInfo(
            c=0,
            actual_ap=out_ap.ap,
            indirect_dim_max_index=IMG2_ELEMS,
            offset_expr=[
                mybir.DynamicAccessPatternOffsetExpr(
                    coef=1,
                    aff_expr=mybir.DynamicAccessPatternOffsetExprAffExpr(
                        kind="IndirectArgId", arg_id=1),
                )
            ],
        )
        in_l.append(off_l[0])
        return eng.add_instruction(
            mybir.InstDMACopy(
                name=nc.get_next_instruction_name(),
                queue="qPoolDynamic",
                mode="Copy",
                ins=in_l,
                outs=out_l,
                oob_is_err=True,
                cce_op=mybir.AluOpType.bypass,
            ))

    with tile.TileContext(nc) as tc, ExitStack() as ctx:
        const = ctx.enter_context(tc.tile_pool(name="const", bufs=1))
        work = ctx.enter_context(tc.tile_pool(name="work", bufs=2))
        small = ctx.enter_context(tc.tile_pool(name="small", bufs=2))
        ps_f = ctx.enter_context(tc.tile_pool(name="ps_f", bufs=1, space="PSUM"))
        ps_hl = ctx.enter_context(tc.tile_pool(name="ps_hl", bufs=1, space="PSUM"))
        ps_hg = ctx.enter_context(tc.tile_pool(name="ps_hg", bufs=2, space="PSUM"))
        ps_g = ctx.enter_context(tc.tile_pool(name="ps_g", bufs=2, space="PSUM"))

        # ---- constants ----
        xt_sb = const.tile([128, 4, NS], F32, tag="xt")
        nc.sync.dma_start(xt_sb[:], xt_d.ap())
        wloc_sb = const.tile([128, 4, 2], F32, tag="wloc")
        nc.sync.dma_start(wloc_sb[:], wloc_d.ap())
        wwin_sb = const.tile([128, 32, HG], BF16, tag="wwin")
        for gi in range(4):
            sl = slice(gi * 8, gi * 8 + 8)
            nc.scalar.dma_start(wwin_sb[:, sl, :], wwin_d.ap()[:, sl, :])
        wgl_sb = const.tile([128, 2 * G], BF16, tag="wgl")
        nc.sync.dma_start(wgl_sb[:], wgl_d.ap())
        wgs_sb = wgl_sb[:, 0:G]
        wls_sb = wgl_sb[:, G:2 * G]
        whl_sb = const.tile([2, HL], BF16, tag="whl")
        nc.sync.dma_start(whl_sb[:], whl_d.ap())
        csm = const.tile([NS, 5], F32, tag="csm")
        nc.sync.dma_start(csm[:], csm_d.ap())
        bloc_sb = csm[:, 0:2]
        sampb_sb = csm[:, 2:3]
        ccol = const.tile([128, 4], F32, tag="ccol")
        nc.sync.dma_start(ccol[:], ccol_d.ap())
        bhg_sb = ccol[:, 0:1]
        bhl_sb = ccol[:, 1:2]
        bloc2_sb = ccol[0:2, 2:3]
        bg_sb = const.tile([1, G + NS], F32, tag="bg")
        nc.sync.dma_start(bg_sb[:], bg_d.ap())
        ones_sb = bg_sb[0:1, G:G + NS]

        loop_cm = (tc.For_i(0, hw_loop, 1, staggered_reset=stag)
                   if hw_loop else nullcontext())

        def stage_front(st):
            # ---- loc sample-major (fp32 index chain) ----
            ploc = ps_f.tile([NS, 2], F32, tag="ploc")
            for k in range(4):
                nc.tensor.matmul(ploc[:], xt_sb[:, k, :], wloc_sb[:, k, :],
                                 start=(k == 0), stop=(k == 3))
            loc_sb = small.tile([NS, 2], F32, tag="loc")
            nc.vector.tensor_tensor(loc_sb[:], ploc[:], bloc_sb, op=Alu.add)
            nc.vector.tensor_scalar(loc_sb[:], loc_sb[:], 1.0, -1.0,
                                    op0=Alu.min, op1=Alu.max)
            lr_sb = small.tile([NS, 2], F32, tag="lr")
            nc.vector.tensor_scalar(lr_sb[:], loc_sb[:], 112.0, 112.0,
                                    op0=Alu.mult, op1=Alu.add)
            nc.vector.tensor_scalar(lr_sb[:], lr_sb[:], BIG, BIG,
                                    op0=Alu.add, op1=Alu.subtract)
            band_sb = small.tile([NS, 1], F32, tag="band")
            nc.vector.tensor_scalar(band_sb[:], lr_sb[:, 1:2], 1.0 / BSTRIDE, 0.51,
                                    op0=Alu.mult, op1=Alu.add)
            nc.vector.tensor_scalar(band_sb[:], band_sb[:], BIG, BIG,
                                    op0=Alu.add, op1=Alu.subtract)
            # A = sampb' + (band+1)*(BANDE-32) + l0*96 + l1
            a_sb = small.tile([NS, NH], F32, tag="abase")
            t_sb = small.tile([NS, 1], F32, tag="tmp")
            nc.vector.tensor_scalar(t_sb[:], band_sb[:], float(BANDE - BSTRIDE),
                                    None, op0=Alu.mult)
            nc.vector.tensor_tensor(t_sb[:], t_sb[:], sampb_sb, op=Alu.add)
            nc.vector.tensor_scalar(a_sb[:, 0:1], lr_sb[:, 0:1], float(BW), None,
                                    op0=Alu.mult)
            nc.vector.tensor_tensor(a_sb[:, 0:1], a_sb[:, 0:1], t_sb[:], op=Alu.add)
            nc.vector.tensor_tensor(a_sb[:, 0:1], a_sb[:, 0:1], lr_sb[:, 1:2],
                                    op=Alu.add)
            idx_sb = small.tile([NS, 1], dt.int32, tag="idx")
            nc.vector.tensor_copy(idx_sb[:], a_sb[:, 0:1])
            st["idx"] = idx_sb
            st["loc"] = loc_sb

            # ---- loc unit-major -> hl ----
            plocT = ps_f.tile([2, NS], F32, tag="plocT")
            for k in range(4):
                nc.tensor.matmul(plocT[:], wloc_sb[:, k, :], xt_sb[:, k, :],
                                 start=(k == 0), stop=(k == 3))
            locT_sb = small.tile([2, NS], BF16, tag="locT")
            nc.scalar.activation(locT_sb[:], plocT[:],
                                 mybir.ActivationFunctionType.Identity,
                                 bias=bloc2_sb)
            nc.vector.tensor_scalar(locT_sb[:], locT_sb[:], 1.0, -1.0,
                                    op0=Alu.min, op1=Alu.max)
            phl = ps_hl.tile([HL, NS], F32, tag="phl")
            nc.tensor.matmul(phl[:], whl_sb[:], locT_sb[:], start=True, stop=True)
            hlT_sb = small.tile([HL, NS], BF16, tag="hlT")
            nc.scalar.activation(hlT_sb[:], phl[:], Relu, bias=bhl_sb)
            st["hlT"] = hlT_sb

        def stage_gx(st):
            # ---- one gather (whole 64x96 window, junk cols included) ----
            gbuf = work.tile([NS, NH * HRUN], BF16, tag="gbuf")
            st["gbuf"] = gbuf
            if do_gather:
                indirect_gather_elem(gbuf[:], st["idx"][:])

        def stage_cx(st):
            # ---- compact (drop junk cols; DVE + gpsimd halves), then
            # transpose via TWO xbars in parallel on the two HWDGE engines --
            gv = st["gbuf"][:].rearrange("p (r w) -> p r w", w=BW)
            win_c = work.tile([NS, 64, 64], BF16, tag="winc")
            nc.vector.tensor_copy(win_c[:, 0:32, :], gv[:, 0:32, 0:64])
            nc.gpsimd.tensor_copy(win_c[:, 32:64, :], gv[:, 32:64, 0:64])
            wf = win_c[:].rearrange("p r c -> p (r c)")
            rhsTa = work.tile([128, 16, NS], BF16, tag="rhsTa")
            rhsTb = work.tile([128, 16, NS], BF16, tag="rhsTb")
            if do_xbar:
                nc.sync.dma_start(rhsTa[:], wf[:, 0:2048], transpose=True)
                nc.scalar.dma_start(rhsTb[:], wf[:, 2048:4096], transpose=True)
            st["rhsT"] = (rhsTa, rhsTb)

        def stage_tail(st):
            # ---- window matmuls -> hg ----
            phg = ps_hg.tile([HG, NS], F32, tag="phg")
            if do_win:
                for c in range(32):
                    rt = st["rhsT"][c // 16]
                    nc.tensor.matmul(phg[:], wwin_sb[:, c, :],
                                     rt[:, c % 16, :],
                                     start=(c == 0), stop=(c == 31))
            if not (do_win and do_tail):
                return
            hgT_sb = work.tile([HG, NS], BF16, tag="hgT")
            nc.scalar.activation(hgT_sb[:], phg[:], Relu, bias=bhg_sb)

            # ---- g = relu(W_gs^T hg + W_ls^T hl + bias), sample-major ----
            pg = ps_g.tile([NS, G], F32, tag="pg")
            nc.tensor.matmul(pg[:], hgT_sb[:], wgs_sb, start=True, stop=False)
            nc.tensor.matmul(pg[:], st["hlT"][:], wls_sb, start=False, stop=False)
            nc.tensor.matmul(pg[:], ones_sb, bg_sb[0:1, 0:G], start=False, stop=True)
            g_sb = work.tile([NS, G], F32, tag="g")
            nc.vector.tensor_scalar(g_sb[:], pg[:], 0.0, None, op0=Alu.max)
            nc.sync.dma_start(out_d.ap(), g_sb[:])

            if debug:
                nc.sync.dma_start(dbg_loc.ap(), st["loc"][:])
                nc.sync.dma_start(dbg_idx.ap(), st["idx"][:])
                gf = work.tile([NS, NH * HRUN], F32, tag="gf")
                nc.vector.tensor_copy(gf[:], st["gbuf"][:])
                nc.sync.dma_start(dbg_g.ap(), gf[:])
                hgf = work.tile([HG, NS], F32, tag="hgf")
                nc.vector.tensor_copy(hgf[:], hgT_sb[:])
                nc.sync.dma_start(dbg_hg.ap(), hgf[:])

        # Stage-interleaved emission (1-deep software pipeline): engines
        # execute their queues in emission order, so copy k+1's cheap front
        # must be enqueued BEFORE copy k's tail to avoid head-of-line
        # blocking behind the gather/xbar latency chain.
        with loop_cm:
            ncopies = loop_n * unroll
            states = [dict() for _ in range(ncopies)]
            if do_front:
                for st in states:
                    stage_front(st)
                    stage_gx(st)
                for st in states:
                    stage_cx(st)
                for st in states:
                    stage_tail(st)

    nc.compile()
    return nc


def _host_prep(inputs):
    """Build the per-core in_maps (pure layout transforms of the inputs)."""
    X = np.ascontiguousarray(np.asarray(inputs["output"], dtype=_F32))
    img = np.asarray(inputs["inputs"], dtype=_F32)[..., 0]
    W_loc = np.asarray(inputs["W_loc"], dtype=_F32)
    b_loc = np.asarray(inputs["b_loc"], dtype=_F32)
    W_hl = np.asarray(inputs["W_hl"], dtype=_F32)
    b_hl = np.asarray(inputs["b_hl"], dtype=_F32)
    W_gs = np.asarray(inputs["W_gs"], dtype=_F32)
    b_gs = np.asarray(inputs["b_gs"], dtype=_F32)
    W_ls = np.asarray(inputs["W_ls"], dtype=_F32)
    b_ls = np.asarray(inputs["b_ls"], dtype=_F32)
    b_hg = np.asarray(inputs["b_hg"], dtype=_F32)

    wwin_g = _build_wwin_chunks(inputs["W_hg"])     # bf16 [128, 32, HG]

    # padded [B, 288, 320] bf16 -> bands [B, 8, 288, 96]
    padded = np.pad(img, ((0, 0), (PAD, PAD), (PAD, PADC_R))).astype(_BF16)
    bands = np.stack([padded[:, :, BSTRIDE * k:BSTRIDE * k + BW]
                      for k in range(NBAND)], axis=1)

    # xt[p, k, s] = X[s, 128k + p]
    xt = np.ascontiguousarray(
        X.reshape(NS * N_CORES, 4, 128).transpose(2, 1, 0))  # [128, 4, B]
    wloc = np.ascontiguousarray(
        W_loc.reshape(4, 128, 2).transpose(1, 0, 2))          # [128, 4, 2]
    wgl = np.concatenate([W_gs, W_ls], axis=1).astype(_BF16)  # [128, 512]
    whl = W_hl.astype(_BF16)                                  # [2, 128]

    s = np.arange(NS, dtype=np.float64)
    csm = np.zeros((NS, 5), _F32)
    csm[:, 0:2] = b_loc[None, :]
    csm[:, 2] = (s * SAMPE - (BANDE - BSTRIDE)).astype(_F32)
    csm[:, 3] = 1.0
    ccol = np.zeros((128, 4), _F32)
    ccol[:, 0] = b_hg
    ccol[:, 1] = b_hl
    ccol[0:2, 2] = b_loc
    bg = np.zeros((1, G + NS), _F32)
    bg[0, 0:G] = b_gs + b_ls
    bg[0, G:] = 1.0

    in_maps = []
    for c in range(N_CORES):
        sl = slice(c * NS, (c + 1) * NS)
        imgc = np.concatenate(
            [bands[sl].reshape(-1),
             np.zeros(IMG_ROWS * 1024 - NS * SAMPE, _BF16)]).reshape(IMG_ROWS, 1024)
        in_maps.append({
            "img": imgc,
            "xt": np.ascontiguousarray(xt[:, :, sl]),
            "wloc": wloc,
            "wwin": wwin_g,
            "wgl": wgl,
            "whl": whl,
            "csm": csm,
            "ccol": ccol,
            "bg": bg,
        })
    return in_maps


def kernel(**inputs) -> np.ndarray:
    from concourse.bass_utils import run_bass_kernel_spmd

    if "nc" not in _CACHE:
        _CACHE["nc"] = _build_nc()
    nc = _CACHE["nc"]
    in_maps = _host_prep(inputs)
    res = run_bass_kernel_spmd(nc, in_maps, core_ids=list(range(N_CORES)))
    out = np.concatenate([res.results[c]["out"] for c in range(N_CORES)], axis=0)
    return out.astype(np.float32)


# revision 21
# speedup vs baseline: 1.0315x; 1.0186x over previous
"""Trainium2 Bass kernel for nn_AttentionControl (recurrent attention glimpse
network step, eval mode).

Contract: kernel(**inputs) takes the FULL unsharded inputs (B=512) and
returns the full [512, 256] output. Pure data parallel across 8 NeuronCores
(64 samples each). All compute runs on device; the host only reshapes/pads
inputs (data-independent layout prep) and folds the constant linear ops
(crop-select + bilinear antialias resize) into the dense weights.

v2 design (per core, per body) — minimal instruction count, no PE transposes:
  1. loc both ways via flipped matmuls on const X^T chunks:
     sample-major ploc [64,2] (for the exact fp32 index chain) and
     unit-major plocT [2,64] (feeds the hl matmul directly).
  2. fp32 index chain on DVE: clip, lr=RNE(112*loc+112) via +2^23,
     band=floor(l1/32) via +0.51/RNE, A = sample_base + band*(BANDE-32)
     + l0*96 + l1 (all exact in f32, < 2^24).
  3. ONE indirect element-granularity gather per half: partition p = sample,
     reads 32 consecutive 96-wide band rows (whole half-window incl. 33%
     junk cols) as one contiguous bf16 run. The image is stored banded
     (8 col-bands of width 96 at stride 32) in bf16.
  4. ONE xbar DMA-transpose per half: [64, 3072] -> [128, 24, 64] bf16 =
     pixel-major chunks. Junk columns are handled by ZERO rows folded into
     the weights (wwin expanded 4096 -> 6144 gathered positions).
  5. 48 accumulating bf16 matmuls (lhsT = folded weights [128,128],
     rhs = xbar chunks [128,64]) -> phg [HG, 64] in PSUM.
  6. hgT/hlT via scalar-engine Relu+bias (per-partition bias), then g
     sample-major via flipped matmuls (lhsT = hgT/hlT [128,64], rhs =
     W_gs/W_ls [128,256]) + K=1 ones-matmul adding the g bias; Relu on DVE;
     direct [64,256] f32 DMA out. No transposes anywhere in the tail.
"""
import numpy as np
import ml_dtypes

# ---------------- problem constants (hardcoded per contract) ----------------
B = 512
STATE = 512
S = 224
G = 256
HG = 128
HL = 128
TSB = 768
N_CORES = 8
NS = B // N_CORES            # samples per core = 64
PAD = 32                     # window pad (window = 64x64 around loc)
PADC_R = 64                  # right col pad so the last band (start 224) fits
PR = S + 2 * PAD             # padded rows = 288
BW = 96                      # band width
BSTRIDE = 32                 # band column stride
NBAND = 8                    # bands at column starts 0, 32, ..., 224
BANDE = PR * BW              # elements per band = 27648
SAMPE = NBAND * BANDE        # elements per sample = 221184
NH = 2                       # window halves (rows 0-31 / 32-63)
HROWS = 32                   # band rows per half
HRUN = HROWS * BW            # gathered elements per half = 3072
NCH = HRUN // 128            # xbar chunks per half = 24
IMG2_ELEMS = NS * SAMPE + 2048   # + tail pad
IMG_ROWS = IMG2_ELEMS // 1024    # img declared [IMG_ROWS, 1024] bf16 so the
                                 # src AP's innermost run is wide (desc sizing)

_F32 = np.float32
_BF16 = ml_dtypes.bfloat16


def _resize_weight_mat(d, n=16):
    """jax.image.resize 'bilinear' (triangle kernel, antialias=True) weight
    matrix [d, n]; resized = w.T @ x @ w for a [d, d] input."""
    scale = _F32(n / d)
    inv_scale = _F32(1.0) / scale
    kernel_scale = np.maximum(inv_scale, _F32(1.0))
    sample_f = (np.arange(n, dtype=_F32) + _F32(0.5)) * inv_scale - _F32(0.5)
    x = np.abs(sample_f[None, :] - np.arange(d, dtype=_F32)[:, None]) / kernel_scale
    w = np.maximum(_F32(0), _F32(1) - np.abs(x)).astype(_F32)
    total = w.sum(axis=0, keepdims=True, dtype=_F32)
    w = np.where(np.abs(total) > 1000.0 * np.finfo(_F32).eps,
                 (w / np.where(total != 0, total, 1)).astype(_F32), 0.0).astype(_F32)
    keep = (sample_f >= -0.5) & (sample_f <= d - 0.5)
    return np.where(keep[None, :], w, 0.0).astype(_F32)


def _build_wwin(W_hg):
    """Fold crop-select + resize + W_hg into Wwin [4096, 128] acting on the
    flattened 64x64 window."""
    W = np.asarray(W_hg, dtype=np.float64)
    L = np.zeros((4096, TSB), dtype=np.float64)
    for i in range(16):
        for j in range(16):
            L[(24 + i) * 64 + (24 + j), i * 16 + j] = 1.0
    w32 = _resize_weight_mat(32).astype(np.float64)
    blk32 = np.einsum("ri,cj->rcij", w32, w32).reshape(32, 32, 256)
    for r in range(32):
        for c in range(32):
            L[(16 + r) * 64 + (16 + c), 256:512] = blk32[r, c]
    w64 = _resize_weight_mat(64).astype(np.float64)
    blk64 = np.einsum("ri,cj->rcij", w64, w64).reshape(64, 64, 256)
    for r in range(64):
        for c in range(64):
            L[r * 64 + c, 512:768] = blk64[r, c]
    return (L @ W).astype(_F32)  # [4096, 128]


def _build_wwin_chunks(W_hg):
    """Expand Wwin to the gathered-position basis [2*HRUN, HG] (junk cols
    q%96 >= 64 get zero weight), chunked: [128, 48, HG]."""
    wwin = _build_wwin(W_hg)                       # [4096, 128]
    wg = np.zeros((NH * HRUN, HG), np.float64)
    w4 = wwin.reshape(64, 64, HG)
    for r in range(64):
        wg[r * BW: r * BW + 64] = w4[r]
    wg = wg.reshape(NH * NCH, 128, HG).transpose(1, 0, 2)  # [128, 48, HG]
    return np.ascontiguousarray(wg.astype(_BF16))


# ---------------------------------------------------------------------------
# Bass program (built once, cached)
# ---------------------------------------------------------------------------
_CACHE = {}

BIG = float(2.0 ** 23)


def _build_nc(debug=False, loop_n=1, hw_loop=0, unroll=1,
              do_front=True, do_gather=True, do_xbar=True, do_win=True,
              do_tail=True, stag=False):
    from contextlib import ExitStack, nullcontext
    import concourse.bass as bass
    import concourse.mybir as mybir
    import concourse.tile as tile
    from concourse import bacc

    dt = mybir.dt
    nc = bacc.Bacc("TRN2", target_bir_lowering=False, debug=False,
                   num_devices=N_CORES)

    F32 = dt.float32
    BF16 = dt.bfloat16
    Relu = mybir.ActivationFunctionType.Relu
    Alu = mybir.AluOpType

    # ---- DRAM I/O ----
    img = nc.dram_tensor("img", [IMG_ROWS, 1024], BF16, kind="ExternalInput")
    xt_d = nc.dram_tensor("xt", [128, 4, NS], F32, kind="ExternalInput")
    wloc_d = nc.dram_tensor("wloc", [128, 4, 2], F32, kind="ExternalInput")
    wwin_d = nc.dram_tensor("wwin", [128, NH * NCH, HG], BF16, kind="ExternalInput")
    wgl_d = nc.dram_tensor("wgl", [128, 2 * G], BF16, kind="ExternalInput")  # wgs|wls
    whl_d = nc.dram_tensor("whl", [2, HL], BF16, kind="ExternalInput")
    csm_d = nc.dram_tensor("csm", [NS, 5], F32, kind="ExternalInput")   # bloc[2], sampb, ones, pad
    ccol_d = nc.dram_tensor("ccol", [128, 4], F32, kind="ExternalInput")  # bhg, bhl, bloc2 cols 0/1 (rows 0-1)
    bg_d = nc.dram_tensor("bg", [1, G + NS], F32, kind="ExternalInput")  # bias_g | ones
    out_d = nc.dram_tensor("out", [NS, G], F32, kind="ExternalOutput")
    if debug:
        dbg_loc = nc.dram_tensor("dbg_loc", [NS, 2], F32, kind="ExternalOutput")
        dbg_idx = nc.dram_tensor("dbg_idx", [NS, 1], dt.int32, kind="ExternalOutput")
        dbg_g = nc.dram_tensor("dbg_g", [NS, NH * HRUN], F32, kind="ExternalOutput")
        dbg_hg = nc.dram_tensor("dbg_hg", [HG, NS], F32, kind="ExternalOutput")

    def indirect_gather_elem(out_ap, idx_ap):
        # per-partition contiguous element-granularity gather from img flat;
        # probed HW semantics: offsets [P,1] int32, dest [P,F], each partition
        # reads F contiguous elements from flat[idx[p]].
        eng = nc.gpsimd
        out_l = eng.lower_ap_dma(out_ap, for_indirect_dma=True)
        in_l = eng.lower_ap_dma(img.ap(), for_indirect_dma=True)
        off_l = eng.lower_ap_dma(idx_ap)
        assert len(out_l) == 1 and len(in_l) == 1 and len(off_l) == 1
        in_l[0].dynamic_ap_info = mybir.DynamicAccessPatternInfo(
            c=0,
            actual_ap=out_ap.ap,
            indirect_dim_max_index=IMG2_ELEMS,
            offset_expr=[
                mybir.DynamicAccessPatternOffsetExpr(
                    coef=1,
                    aff_expr=mybir.DynamicAccessPatternOffsetExprAffExpr(
                        kind="IndirectArgId", arg_id=1),
                )
            ],
        )
        in_l.append(off_l[0])
        return eng.add_instruction(
            mybir.InstDMACopy(
                name=nc.get_next_instruction_name(),
                queue="qPoolDynamic",
                mode="Copy",
                ins=in_l,
                outs=out_l,
                oob_is_err=True,
                cce_op=mybir.AluOpType.bypass,
            ))

    with tile.TileContext(nc) as tc, ExitStack() as ctx:
        const = ctx.enter_context(tc.tile_pool(name="const", bufs=1))
        work = ctx.enter_context(tc.tile_pool(name="work", bufs=2))
        small = ctx.enter_context(tc.tile_pool(name="small", bufs=2))
        ps_f = ctx.enter_context(tc.tile_pool(name="ps_f", bufs=1, space="PSUM"))
        ps_hl = ctx.enter_context(tc.tile_pool(name="ps_hl", bufs=1, space="PSUM"))
        ps_hg = ctx.enter_context(tc.tile_pool(name="ps_hg", bufs=2, space="PSUM"))
        ps_g = ctx.enter_context(tc.tile_pool(name="ps_g", bufs=2, space="PSUM"))

        # ---- constants ----
        xt_sb = const.tile([128, 4, NS], F32, tag="xt")
        nc.sync.dma_start(xt_sb[:], xt_d.ap())
        wloc_sb = const.tile([128, 4, 2], F32, tag="wloc")
        nc.sync.dma_start(wloc_sb[:], wloc_d.ap())
        wwin_sb = const.tile([128, NH * NCH, HG], BF16, tag="wwin")
        for gi in range(4):
            sl = slice(gi * 12, gi * 12 + 12)
            nc.scalar.dma_start(wwin_sb[:, sl, :], wwin_d.ap()[:, sl, :])
        wgl_sb = const.tile([128, 2 * G], BF16, tag="wgl")
        nc.sync.dma_start(wgl_sb[:], wgl_d.ap())
        wgs_sb = wgl_sb[:, 0:G]
        wls_sb = wgl_sb[:, G:2 * G]
        whl_sb = const.tile([2, HL], BF16, tag="whl")
        nc.sync.dma_start(whl_sb[:], whl_d.ap())
        csm = const.tile([NS, 5], F32, tag="csm")
        nc.sync.dma_start(csm[:], csm_d.ap())
        bloc_sb = csm[:, 0:2]
        sampb_sb = csm[:, 2:3]
        ccol = const.tile([128, 4], F32, tag="ccol")
        nc.sync.dma_start(ccol[:], ccol_d.ap())
        bhg_sb = ccol[:, 0:1]
        bhl_sb = ccol[:, 1:2]
        bloc2_sb = ccol[0:2, 2:3]
        bg_sb = const.tile([1, G + NS], F32, tag="bg")
        nc.sync.dma_start(bg_sb[:], bg_d.ap())
        ones_sb = bg_sb[0:1, G:G + NS]

        loop_cm = (tc.For_i(0, hw_loop, 1, staggered_reset=stag)
                   if hw_loop else nullcontext())

        def stage_front(st):
            # ---- loc sample-major (fp32 index chain) ----
            ploc = ps_f.tile([NS, 2], F32, tag="ploc")
            for k in range(4):
                nc.tensor.matmul(ploc[:], xt_sb[:, k, :], wloc_sb[:, k, :],
                                 start=(k == 0), stop=(k == 3))
            loc_sb = small.tile([NS, 2], F32, tag="loc")
            nc.vector.tensor_tensor(loc_sb[:], ploc[:], bloc_sb, op=Alu.add)
            nc.vector.tensor_scalar(loc_sb[:], loc_sb[:], 1.0, -1.0,
                                    op0=Alu.min, op1=Alu.max)
            lr_sb = small.tile([NS, 2], F32, tag="lr")
            nc.vector.tensor_scalar(lr_sb[:], loc_sb[:], 112.0, 112.0,
                                    op0=Alu.mult, op1=Alu.add)
            nc.vector.tensor_scalar(lr_sb[:], lr_sb[:], BIG, BIG,
                                    op0=Alu.add, op1=Alu.subtract)
            band_sb = small.tile([NS, 1], F32, tag="band")
            nc.vector.tensor_scalar(band_sb[:], lr_sb[:, 1:2], 1.0 / BSTRIDE, 0.51,
                                    op0=Alu.mult, op1=Alu.add)
            nc.vector.tensor_scalar(band_sb[:], band_sb[:], BIG, BIG,
                                    op0=Alu.add, op1=Alu.subtract)
            # A = sampb' + (band+1)*(BANDE-32) + l0*96 + l1
            a_sb = small.tile([NS, NH], F32, tag="abase")
            t_sb = small.tile([NS, 1], F32, tag="tmp")
            nc.vector.tensor_scalar(t_sb[:], band_sb[:], float(BANDE - BSTRIDE),
                                    None, op0=Alu.mult)
            nc.vector.tensor_tensor(t_sb[:], t_sb[:], sampb_sb, op=Alu.add)
            nc.vector.tensor_scalar(a_sb[:, 0:1], lr_sb[:, 0:1], float(BW), None,
                                    op0=Alu.mult)
            nc.vector.tensor_tensor(a_sb[:, 0:1], a_sb[:, 0:1], t_sb[:], op=Alu.add)
            nc.vector.tensor_tensor(a_sb[:, 0:1], a_sb[:, 0:1], lr_sb[:, 1:2],
                                    op=Alu.add)
            idx_sb = small.tile([NS, 1], dt.int32, tag="idx")
            nc.vector.tensor_copy(idx_sb[:], a_sb[:, 0:1])
            st["idx"] = idx_sb
            st["loc"] = loc_sb

            # ---- loc unit-major -> hl ----
            plocT = ps_f.tile([2, NS], F32, tag="plocT")
            for k in range(4):
                nc.tensor.matmul(plocT[:], wloc_sb[:, k, :], xt_sb[:, k, :],
                                 start=(k == 0), stop=(k == 3))
            locT_sb = small.tile([2, NS], BF16, tag="locT")
            nc.scalar.activation(locT_sb[:], plocT[:],
                                 mybir.ActivationFunctionType.Identity,
                                 bias=bloc2_sb)
            nc.vector.tensor_scalar(locT_sb[:], locT_sb[:], 1.0, -1.0,
                                    op0=Alu.min, op1=Alu.max)
            phl = ps_hl.tile([HL, NS], F32, tag="phl")
            nc.tensor.matmul(phl[:], whl_sb[:], locT_sb[:], start=True, stop=True)
            hlT_sb = small.tile([HL, NS], BF16, tag="hlT")
            nc.scalar.activation(hlT_sb[:], phl[:], Relu, bias=bhl_sb)
            st["hlT"] = hlT_sb

        def stage_gx(st):
            # ---- one gather (whole 64x96 window, junk cols included) ----
            gbuf = work.tile([NS, NH * HRUN], BF16, tag="gbuf")
            st["gbuf"] = gbuf
            if do_gather:
                indirect_gather_elem(gbuf[:], st["idx"][:])

        def stage_cx(st):
            # ---- ONE xbar DMA-transpose: [64, 6144] -> [128, 48, 64]
            # pixel-major chunks (junk cols ride along; zero weight rows
            # kill them in the matmul) ----
            rhsT = work.tile([128, NH * NCH, NS], BF16, tag="rhsT")
            if do_xbar:
                nc.sync.dma_start(rhsT[:], st["gbuf"][:], transpose=True)
            st["rhsT"] = rhsT

        def stage_tail(st):
            # ---- window matmuls -> hg ----
            phg = ps_hg.tile([HG, NS], F32, tag="phg")
            if do_win:
                for c in range(NH * NCH):
                    nc.tensor.matmul(phg[:], wwin_sb[:, c, :],
                                     st["rhsT"][:, c, :],
                                     start=(c == 0), stop=(c == NH * NCH - 1))
            if not (do_win and do_tail):
                return
            hgT_sb = work.tile([HG, NS], BF16, tag="hgT")
            nc.scalar.activation(hgT_sb[:], phg[:], Relu, bias=bhg_sb)

            # ---- g = relu(W_gs^T hg + W_ls^T hl + bias), sample-major ----
            pg = ps_g.tile([NS, G], F32, tag="pg")
            nc.tensor.matmul(pg[:], hgT_sb[:], wgs_sb, start=True, stop=False)
            nc.tensor.matmul(pg[:], st["hlT"][:], wls_sb, start=False, stop=False)
            nc.tensor.matmul(pg[:], ones_sb, bg_sb[0:1, 0:G], start=False, stop=True)
            g_sb = work.tile([NS, G], F32, tag="g")
            nc.scalar.activation(g_sb[:], pg[:], Relu)
            nc.sync.dma_start(out_d.ap(), g_sb[:])

            if debug:
                nc.sync.dma_start(dbg_loc.ap(), st["loc"][:])
                nc.sync.dma_start(dbg_idx.ap(), st["idx"][:])
                gf = work.tile([NS, NH * HRUN], F32, tag="gf")
                nc.vector.tensor_copy(gf[:], st["gbuf"][:])
                nc.sync.dma_start(dbg_g.ap(), gf[:])
                hgf = work.tile([HG, NS], F32, tag="hgf")
                nc.vector.tensor_copy(hgf[:], hgT_sb[:])
                nc.sync.dma_start(dbg_hg.ap(), hgf[:])

        # Stage-interleaved emission (1-deep software pipeline): engines
        # execute their queues in emission order, so copy k+1's cheap front
        # must be enqueued BEFORE copy k's tail to avoid head-of-line
        # blocking behind the gather/xbar latency chain.
        with loop_cm:
            ncopies = loop_n * unroll
            states = [dict() for _ in range(ncopies)]
            if do_front:
                for st in states:
                    stage_front(st)
                    stage_gx(st)
                for st in states:
                    stage_cx(st)
                for st in states:
                    stage_tail(st)

    nc.compile()
    return nc


def _host_prep(inputs):
    """Build the per-core in_maps (pure layout transforms of the inputs)."""
    X = np.ascontiguousarray(np.asarray(inputs["output"], dtype=_F32))
    img = np.asarray(inputs["inputs"], dtype=_F32)[..., 0]
    W_loc = np.asarray(inputs["W_loc"], dtype=_F32)
    b_loc = np.asarray(inputs["b_loc"], dtype=_F32)
    W_hl = np.asarray(inputs["W_hl"], dtype=_F32)
    b_hl = np.asarray(inputs["b_hl"], dtype=_F32)
    W_gs = np.asarray(inputs["W_gs"], dtype=_F32)
    b_gs = np.asarray(inputs["b_gs"], dtype=_F32)
    W_ls = np.asarray(inputs["W_ls"], dtype=_F32)
    b_ls = np.asarray(inputs["b_ls"], dtype=_F32)
    b_hg = np.asarray(inputs["b_hg"], dtype=_F32)

    wwin_g = _build_wwin_chunks(inputs["W_hg"])     # bf16 [128, 32, HG]

    # padded [B, 288, 320] bf16 -> bands [B, 8, 288, 96]
    padded = np.pad(img, ((0, 0), (PAD, PAD), (PAD, PADC_R))).astype(_BF16)
    bands = np.stack([padded[:, :, BSTRIDE * k:BSTRIDE * k + BW]
                      for k in range(NBAND)], axis=1)

    # xt[p, k, s] = X[s, 128k + p]
    xt = np.ascontiguousarray(
        X.reshape(NS * N_CORES, 4, 128).transpose(2, 1, 0))  # [128, 4, B]
    wloc = np.ascontiguousarray(
        W_loc.reshape(4, 128, 2).transpose(1, 0, 2))          # [128, 4, 2]
    wgl = np.concatenate([W_gs, W_ls], axis=1).astype(_BF16)  # [128, 512]
    whl = W_hl.astype(_BF16)                                  # [2, 128]

    s = np.arange(NS, dtype=np.float64)
    csm = np.zeros((NS, 5), _F32)
    csm[:, 0:2] = b_loc[None, :]
    csm[:, 2] = (s * SAMPE - (BANDE - BSTRIDE)).astype(_F32)
    csm[:, 3] = 1.0
    ccol = np.zeros((128, 4), _F32)
    ccol[:, 0] = b_hg
    ccol[:, 1] = b_hl
    ccol[0:2, 2] = b_loc
    bg = np.zeros((1, G + NS), _F32)
    bg[0, 0:G] = b_gs + b_ls
    bg[0, G:] = 1.0

    in_maps = []
    for c in range(N_CORES):
        sl = slice(c * NS, (c + 1) * NS)
        imgc = np.concatenate(
            [bands[sl].reshape(-1),
             np.zeros(IMG_ROWS * 1024 - NS * SAMPE, _BF16)]).reshape(IMG_ROWS, 1024)
        in_maps.append({
            "img": imgc,
            "xt": np.ascontiguousarray(xt[:, :, sl]),
            "wloc": wloc,
            "wwin": wwin_g,
            "wgl": wgl,
            "whl": whl,
            "csm": csm,
            "ccol": ccol,
            "bg": bg,
        })
    return in_maps


def kernel(**inputs) -> np.ndarray:
    from concourse.bass_utils import run_bass_kernel_spmd

    if "nc" not in _CACHE:
        _CACHE["nc"] = _build_nc()
    nc = _CACHE["nc"]
    in_maps = _host_prep(inputs)
    res = run_bass_kernel_spmd(nc, in_maps, core_ids=list(range(N_CORES)))
    out = np.concatenate([res.results[c]["out"] for c in range(N_CORES)], axis=0)
    return out.astype(np.float32)


# revision 26
# speedup vs baseline: 1.1083x; 1.0745x over previous
"""Trainium2 Bass kernel for nn_AttentionControl (recurrent attention glimpse
network step, eval mode).

Contract: kernel(**inputs) takes the FULL unsharded inputs (B=512) and
returns the full [512, 256] output. Pure data parallel across 8 NeuronCores
(64 samples each). All compute runs on device; the host only reshapes/pads
inputs (data-independent layout prep) and folds the constant linear ops
(crop-select + bilinear antialias resize) into the dense weights.

v2 design (per core, per body) — minimal instruction count, no PE transposes:
  1. loc both ways via flipped matmuls on const X^T chunks:
     sample-major ploc [64,2] (for the exact fp32 index chain) and
     unit-major plocT [2,64] (feeds the hl matmul directly).
  2. fp32 index chain on DVE: clip, lr=RNE(112*loc+112) via +2^23,
     band=floor(l1/32) via +0.51/RNE, A = sample_base + band*(BANDE-32)
     + l0*96 + l1 (all exact in f32, < 2^24).
  3. ONE indirect element-granularity gather per half: partition p = sample,
     reads 32 consecutive 96-wide band rows (whole half-window incl. 33%
     junk cols) as one contiguous bf16 run. The image is stored banded
     (8 col-bands of width 96 at stride 32) in bf16.
  4. ONE xbar DMA-transpose per half: [64, 3072] -> [128, 24, 64] bf16 =
     pixel-major chunks. Junk columns are handled by ZERO rows folded into
     the weights (wwin expanded 4096 -> 6144 gathered positions).
  5. 48 accumulating bf16 matmuls (lhsT = folded weights [128,128],
     rhs = xbar chunks [128,64]) -> phg [HG, 64] in PSUM.
  6. hgT/hlT via scalar-engine Relu+bias (per-partition bias), then g
     sample-major via flipped matmuls (lhsT = hgT/hlT [128,64], rhs =
     W_gs/W_ls [128,256]) + K=1 ones-matmul adding the g bias; Relu on DVE;
     direct [64,256] f32 DMA out. No transposes anywhere in the tail.
"""
import numpy as np
import ml_dtypes

# ---------------- problem constants (hardcoded per contract) ----------------
B = 512
STATE = 512
S = 224
G = 256
HG = 128
HL = 128
TSB = 768
N_CORES = 8
NS = B // N_CORES            # samples per core = 64
PAD = 32                     # window pad (window = 64x64 around loc)
PADC_R = 64                  # right col pad so the last band (start 224) fits
PR = S + 2 * PAD             # padded rows = 288
BW = 96                      # band width
BSTRIDE = 32                 # band column stride
NBAND = 8                    # bands at column starts 0, 32, ..., 224
BANDE = PR * BW              # elements per band = 27648
SAMPE = NBAND * BANDE        # elements per sample = 221184
NH = 2                       # window halves (rows 0-31 / 32-63)
HROWS = 32                   # band rows per half
HRUN = HROWS * BW            # gathered elements per half = 3072
NCH = HRUN // 128            # xbar chunks per half = 24
IMG2_ELEMS = NS * SAMPE + 2048   # + tail pad
IMG_ROWS = IMG2_ELEMS // 1024    # img declared [IMG_ROWS, 1024] bf16 so the
                                 # src AP's innermost run is wide (desc sizing)

_F32 = np.float32
_BF16 = ml_dtypes.bfloat16


def _resize_weight_mat(d, n=16):
    """jax.image.resize 'bilinear' (triangle kernel, antialias=True) weight
    matrix [d, n]; resized = w.T @ x @ w for a [d, d] input."""
    scale = _F32(n / d)
    inv_scale = _F32(1.0) / scale
    kernel_scale = np.maximum(inv_scale, _F32(1.0))
    sample_f = (np.arange(n, dtype=_F32) + _F32(0.5)) * inv_scale - _F32(0.5)
    x = np.abs(sample_f[None, :] - np.arange(d, dtype=_F32)[:, None]) / kernel_scale
    w = np.maximum(_F32(0), _F32(1) - np.abs(x)).astype(_F32)
    total = w.sum(axis=0, keepdims=True, dtype=_F32)
    w = np.where(np.abs(total) > 1000.0 * np.finfo(_F32).eps,
                 (w / np.where(total != 0, total, 1)).astype(_F32), 0.0).astype(_F32)
    keep = (sample_f >= -0.5) & (sample_f <= d - 0.5)
    return np.where(keep[None, :], w, 0.0).astype(_F32)


def _build_wwin(W_hg):
    """Fold crop-select + resize + W_hg into Wwin [4096, 128] acting on the
    flattened 64x64 window."""
    W = np.asarray(W_hg, dtype=np.float64)
    L = np.zeros((4096, TSB), dtype=np.float64)
    for i in range(16):
        for j in range(16):
            L[(24 + i) * 64 + (24 + j), i * 16 + j] = 1.0
    w32 = _resize_weight_mat(32).astype(np.float64)
    blk32 = np.einsum("ri,cj->rcij", w32, w32).reshape(32, 32, 256)
    for r in range(32):
        for c in range(32):
            L[(16 + r) * 64 + (16 + c), 256:512] = blk32[r, c]
    w64 = _resize_weight_mat(64).astype(np.float64)
    blk64 = np.einsum("ri,cj->rcij", w64, w64).reshape(64, 64, 256)
    for r in range(64):
        for c in range(64):
            L[r * 64 + c, 512:768] = blk64[r, c]
    return (L @ W).astype(_F32)  # [4096, 128]


def _build_wwin_chunks(W_hg):
    """Expand Wwin to the gathered-position basis [2*HRUN, HG] (junk cols
    q%96 >= 64 get zero weight), chunked: [128, 48, HG]."""
    wwin = _build_wwin(W_hg)                       # [4096, 128]
    wg = np.zeros((NH * HRUN, HG), np.float64)
    w4 = wwin.reshape(64, 64, HG)
    for r in range(64):
        wg[r * BW: r * BW + 64] = w4[r]
    wg = wg.reshape(NH * NCH, 128, HG).transpose(1, 0, 2)  # [128, 48, HG]
    return np.ascontiguousarray(wg.astype(_BF16))


# ---------------------------------------------------------------------------
# Bass program (built once, cached)
# ---------------------------------------------------------------------------
_CACHE = {}

BIG = float(2.0 ** 23)


def _build_nc(debug=False, loop_n=1, hw_loop=0, unroll=1,
              do_front=True, do_gather=True, do_xbar=True, do_win=True,
              do_tail=True, stag=False):
    from contextlib import ExitStack, nullcontext
    import concourse.bass as bass
    import concourse.mybir as mybir
    import concourse.tile as tile
    from concourse import bacc

    dt = mybir.dt
    nc = bacc.Bacc("TRN2", target_bir_lowering=False, debug=False,
                   num_devices=N_CORES)

    F32 = dt.float32
    BF16 = dt.bfloat16
    Relu = mybir.ActivationFunctionType.Relu
    Alu = mybir.AluOpType

    # ---- DRAM I/O ----
    img = nc.dram_tensor("img", [IMG_ROWS, 1024], BF16, kind="ExternalInput")
    xt_d = nc.dram_tensor("xt", [128, 4, NS], F32, kind="ExternalInput")
    wloc_d = nc.dram_tensor("wloc", [128, 4, 2], F32, kind="ExternalInput")
    wwin_d = nc.dram_tensor("wwin", [128, NH * NCH, HG], BF16, kind="ExternalInput")
    wgl_d = nc.dram_tensor("wgl", [128, 2 * G], BF16, kind="ExternalInput")  # wgs|wls
    whl_d = nc.dram_tensor("whl", [2, HL], BF16, kind="ExternalInput")
    csm_d = nc.dram_tensor("csm", [NS, 5], F32, kind="ExternalInput")   # bloc[2], sampb, ones, pad
    ccol_d = nc.dram_tensor("ccol", [128, 4], F32, kind="ExternalInput")  # bhg, bhl, bloc2 cols 0/1 (rows 0-1)
    bg_d = nc.dram_tensor("bg", [1, G + NS], F32, kind="ExternalInput")  # bias_g | ones
    out_d = nc.dram_tensor("out", [NS, G], F32, kind="ExternalOutput")
    if debug:
        dbg_loc = nc.dram_tensor("dbg_loc", [NS, 2], F32, kind="ExternalOutput")
        dbg_idx = nc.dram_tensor("dbg_idx", [NS, 1], dt.int32, kind="ExternalOutput")
        dbg_g = nc.dram_tensor("dbg_g", [NS, NH * HRUN], F32, kind="ExternalOutput")
        dbg_hg = nc.dram_tensor("dbg_hg", [HG, NS], F32, kind="ExternalOutput")

    def indirect_gather_elem(out_ap, idx_ap):
        # per-partition contiguous element-granularity gather from img flat;
        # probed HW semantics: offsets [P,1] int32, dest [P,F], each partition
        # reads F contiguous elements from flat[idx[p]].
        eng = nc.gpsimd
        out_l = eng.lower_ap_dma(out_ap, for_indirect_dma=True)
        in_l = eng.lower_ap_dma(img.ap(), for_indirect_dma=True)
        off_l = eng.lower_ap_dma(idx_ap)
        assert len(out_l) == 1 and len(in_l) == 1 and len(off_l) == 1
        in_l[0].dynamic_ap_info = mybir.DynamicAccessPatternInfo(
            c=0,
            actual_ap=out_ap.ap,
            indirect_dim_max_index=IMG2_ELEMS,
            offset_expr=[
                mybir.DynamicAccessPatternOffsetExpr(
                    coef=1,
                    aff_expr=mybir.DynamicAccessPatternOffsetExprAffExpr(
                        kind="IndirectArgId", arg_id=1),
                )
            ],
        )
        in_l.append(off_l[0])
        return eng.add_instruction(
            mybir.InstDMACopy(
                name=nc.get_next_instruction_name(),
                queue="qPoolDynamic",
                mode="Copy",
                ins=in_l,
                outs=out_l,
                oob_is_err=True,
                cce_op=mybir.AluOpType.bypass,
            ))

    with tile.TileContext(nc) as tc, ExitStack() as ctx:
        const = ctx.enter_context(tc.tile_pool(name="const", bufs=1))
        work = ctx.enter_context(tc.tile_pool(name="work", bufs=2))
        small = ctx.enter_context(tc.tile_pool(name="small", bufs=2))
        ps_f = ctx.enter_context(tc.tile_pool(name="ps_f", bufs=1, space="PSUM"))
        ps_hl = ctx.enter_context(tc.tile_pool(name="ps_hl", bufs=1, space="PSUM"))
        ps_hg = ctx.enter_context(tc.tile_pool(name="ps_hg", bufs=2, space="PSUM"))
        ps_g = ctx.enter_context(tc.tile_pool(name="ps_g", bufs=2, space="PSUM"))

        # ---- constants ----
        xt_sb = const.tile([128, 4, NS], F32, tag="xt")
        nc.sync.dma_start(xt_sb[:], xt_d.ap())
        wloc_sb = const.tile([128, 4, 2], F32, tag="wloc")
        nc.sync.dma_start(wloc_sb[:], wloc_d.ap())
        wwin_sb = const.tile([128, NH * NCH, HG], BF16, tag="wwin")
        for gi in range(4):
            sl = slice(gi * 12, gi * 12 + 12)
            nc.scalar.dma_start(wwin_sb[:, sl, :], wwin_d.ap()[:, sl, :])
        wgl_sb = const.tile([128, 2 * G], BF16, tag="wgl")
        nc.sync.dma_start(wgl_sb[:], wgl_d.ap())
        wgs_sb = wgl_sb[:, 0:G]
        wls_sb = wgl_sb[:, G:2 * G]
        whl_sb = const.tile([2, HL], BF16, tag="whl")
        nc.sync.dma_start(whl_sb[:], whl_d.ap())
        csm = const.tile([NS, 5], F32, tag="csm")
        nc.sync.dma_start(csm[:], csm_d.ap())
        bloc_sb = csm[:, 0:2]
        sampb_sb = csm[:, 2:3]
        ccol = const.tile([128, 4], F32, tag="ccol")
        nc.sync.dma_start(ccol[:], ccol_d.ap())
        bhg_sb = ccol[:, 0:1]
        bhl_sb = ccol[:, 1:2]
        bloc2_sb = ccol[0:2, 2:3]
        bg_sb = const.tile([1, G + NS], F32, tag="bg")
        nc.sync.dma_start(bg_sb[:], bg_d.ap())
        ones_sb = bg_sb[0:1, G:G + NS]

        loop_cm = (tc.For_i(0, hw_loop, 1, staggered_reset=stag)
                   if hw_loop else nullcontext())

        def stage_front(st):
            # ---- loc sample-major (fp32 index chain) ----
            ploc = ps_f.tile([NS, 2], F32, tag="ploc")
            for k in range(4):
                nc.tensor.matmul(ploc[:], xt_sb[:, k, :], wloc_sb[:, k, :],
                                 start=(k == 0), stop=(k == 3))
            loc_sb = small.tile([NS, 2], F32, tag="loc")
            nc.vector.tensor_tensor(loc_sb[:], ploc[:], bloc_sb, op=Alu.add)
            nc.vector.tensor_scalar(loc_sb[:], loc_sb[:], 1.0, -1.0,
                                    op0=Alu.min, op1=Alu.max)
            lr_sb = small.tile([NS, 2], F32, tag="lr")
            nc.vector.tensor_scalar(lr_sb[:], loc_sb[:], 112.0, 112.0,
                                    op0=Alu.mult, op1=Alu.add)
            nc.vector.tensor_scalar(lr_sb[:], lr_sb[:], BIG, BIG,
                                    op0=Alu.add, op1=Alu.subtract)
            band_sb = small.tile([NS, 1], F32, tag="band")
            nc.vector.tensor_scalar(band_sb[:], lr_sb[:, 1:2], 1.0 / BSTRIDE, 0.51,
                                    op0=Alu.mult, op1=Alu.add)
            nc.vector.tensor_scalar(band_sb[:], band_sb[:], BIG, BIG,
                                    op0=Alu.add, op1=Alu.subtract)
            # A = sampb' + (band+1)*(BANDE-32) + l0*96 + l1
            a_sb = small.tile([NS, NH], F32, tag="abase")
            t_sb = small.tile([NS, 1], F32, tag="tmp")
            nc.vector.tensor_scalar(t_sb[:], band_sb[:], float(BANDE - BSTRIDE),
                                    None, op0=Alu.mult)
            nc.vector.tensor_tensor(t_sb[:], t_sb[:], sampb_sb, op=Alu.add)
            nc.vector.tensor_scalar(a_sb[:, 0:1], lr_sb[:, 0:1], float(BW), None,
                                    op0=Alu.mult)
            nc.vector.tensor_tensor(a_sb[:, 0:1], a_sb[:, 0:1], t_sb[:], op=Alu.add)
            nc.vector.tensor_tensor(a_sb[:, 0:1], a_sb[:, 0:1], lr_sb[:, 1:2],
                                    op=Alu.add)
            idx_sb = small.tile([NS, 1], dt.int32, tag="idx")
            nc.vector.tensor_copy(idx_sb[:], a_sb[:, 0:1])
            st["idx"] = idx_sb
            st["loc"] = loc_sb

            # ---- loc unit-major -> hl ----
            plocT = ps_f.tile([2, NS], F32, tag="plocT")
            for k in range(4):
                nc.tensor.matmul(plocT[:], wloc_sb[:, k, :], xt_sb[:, k, :],
                                 start=(k == 0), stop=(k == 3))
            locT_sb = small.tile([2, NS], BF16, tag="locT")
            nc.scalar.activation(locT_sb[:], plocT[:],
                                 mybir.ActivationFunctionType.Identity,
                                 bias=bloc2_sb)
            nc.vector.tensor_scalar(locT_sb[:], locT_sb[:], 1.0, -1.0,
                                    op0=Alu.min, op1=Alu.max)
            phl = ps_hl.tile([HL, NS], F32, tag="phl")
            nc.tensor.matmul(phl[:], whl_sb[:], locT_sb[:], start=True, stop=True)
            hlT_sb = small.tile([HL, NS], BF16, tag="hlT")
            nc.scalar.activation(hlT_sb[:], phl[:], Relu, bias=bhl_sb)
            st["hlT"] = hlT_sb

        def stage_gx(st):
            # ---- one gather (whole 64x96 window, junk cols included) ----
            gbuf = work.tile([NS, NH * HRUN], BF16, tag="gbuf")
            st["gbuf"] = gbuf
            if do_gather:
                indirect_gather_elem(gbuf[:], st["idx"][:])

        def stage_cx(st):
            # ---- ONE xbar DMA-transpose: [64, 6144] -> [128, 48, 64]
            # pixel-major chunks (junk cols ride along; zero weight rows
            # kill them in the matmul) ----
            rhsT = work.tile([128, NH * NCH, NS], BF16, tag="rhsT")
            if do_xbar:
                nc.sync.dma_start(rhsT[:], st["gbuf"][:], transpose=True)
            st["rhsT"] = rhsT

        def stage_tail(st):
            # ---- window matmuls -> hg ----
            phg = ps_hg.tile([HG, NS], F32, tag="phg")
            if do_win:
                for c in range(NH * NCH):
                    nc.tensor.matmul(phg[:], wwin_sb[:, c, :],
                                     st["rhsT"][:, c, :],
                                     start=(c == 0), stop=(c == NH * NCH - 1))
            if not (do_win and do_tail):
                return
            hgT_sb = work.tile([HG, NS], BF16, tag="hgT")
            nc.scalar.activation(hgT_sb[:], phg[:], Relu, bias=bhg_sb)

            # ---- g = relu(W_gs^T hg + W_ls^T hl + bias), sample-major ----
            pg = ps_g.tile([NS, G], F32, tag="pg")
            nc.tensor.matmul(pg[:], hgT_sb[:], wgs_sb, start=True, stop=False)
            nc.tensor.matmul(pg[:], st["hlT"][:], wls_sb, start=False, stop=False)
            nc.tensor.matmul(pg[:], ones_sb, bg_sb[0:1, 0:G], start=False, stop=True)
            g_sb = work.tile([NS, G], F32, tag="g")
            nc.scalar.activation(g_sb[:], pg[:], Relu)
            nc.sync.dma_start(out_d.ap(), g_sb[:])

            if debug:
                nc.sync.dma_start(dbg_loc.ap(), st["loc"][:])
                nc.sync.dma_start(dbg_idx.ap(), st["idx"][:])
                gf = work.tile([NS, NH * HRUN], F32, tag="gf")
                nc.vector.tensor_copy(gf[:], st["gbuf"][:])
                nc.sync.dma_start(dbg_g.ap(), gf[:])
                hgf = work.tile([HG, NS], F32, tag="hgf")
                nc.vector.tensor_copy(hgf[:], hgT_sb[:])
                nc.sync.dma_start(dbg_hg.ap(), hgf[:])

        # Stage-interleaved emission (1-deep software pipeline): engines
        # execute their queues in emission order, so copy k+1's cheap front
        # must be enqueued BEFORE copy k's tail to avoid head-of-line
        # blocking behind the gather/xbar latency chain.
        with loop_cm:
            ncopies = loop_n * unroll
            states = [dict() for _ in range(ncopies)]
            if do_front:
                for st in states:
                    stage_front(st)
                    stage_gx(st)
                for st in states:
                    stage_cx(st)
                for st in states:
                    stage_tail(st)

    nc.compile()
    return nc


def _host_prep(inputs):
    """Build the per-core in_maps (pure layout transforms of the inputs)."""
    X = np.ascontiguousarray(np.asarray(inputs["output"], dtype=_F32))
    img = np.asarray(inputs["inputs"], dtype=_F32)[..., 0]
    W_loc = np.asarray(inputs["W_loc"], dtype=_F32)
    b_loc = np.asarray(inputs["b_loc"], dtype=_F32)
    W_hl = np.asarray(inputs["W_hl"], dtype=_F32)
    b_hl = np.asarray(inputs["b_hl"], dtype=_F32)
    W_gs = np.asarray(inputs["W_gs"], dtype=_F32)
    b_gs = np.asarray(inputs["b_gs"], dtype=_F32)
    W_ls = np.asarray(inputs["W_ls"], dtype=_F32)
    b_ls = np.asarray(inputs["b_ls"], dtype=_F32)
    b_hg = np.asarray(inputs["b_hg"], dtype=_F32)

    wwin_g = _build_wwin_chunks(inputs["W_hg"])     # bf16 [128, 32, HG]

    # padded [B, 288, 320] bf16 -> bands [B, 8, 288, 96]
    padded = np.pad(img, ((0, 0), (PAD, PAD), (PAD, PADC_R))).astype(_BF16)
    bands = np.stack([padded[:, :, BSTRIDE * k:BSTRIDE * k + BW]
                      for k in range(NBAND)], axis=1)

    # xt[p, k, s] = X[s, 128k + p]
    xt = np.ascontiguousarray(
        X.reshape(NS * N_CORES, 4, 128).transpose(2, 1, 0))  # [128, 4, B]
    wloc = np.ascontiguousarray(
        W_loc.reshape(4, 128, 2).transpose(1, 0, 2))          # [128, 4, 2]
    wgl = np.concatenate([W_gs, W_ls], axis=1).astype(_BF16)  # [128, 512]
    whl = W_hl.astype(_BF16)                                  # [2, 128]

    s = np.arange(NS, dtype=np.float64)
    csm = np.zeros((NS, 5), _F32)
    csm[:, 0:2] = b_loc[None, :]
    csm[:, 2] = (s * SAMPE - (BANDE - BSTRIDE)).astype(_F32)
    csm[:, 3] = 1.0
    ccol = np.zeros((128, 4), _F32)
    ccol[:, 0] = b_hg
    ccol[:, 1] = b_hl
    ccol[0:2, 2] = b_loc
    bg = np.zeros((1, G + NS), _F32)
    bg[0, 0:G] = b_gs + b_ls
    bg[0, G:] = 1.0

    in_maps = []
    for c in range(N_CORES):
        sl = slice(c * NS, (c + 1) * NS)
        imgc = np.concatenate(
            [bands[sl].reshape(-1),
             np.zeros(IMG_ROWS * 1024 - NS * SAMPE, _BF16)]).reshape(IMG_ROWS, 1024)
        in_maps.append({
            "img": imgc,
            "xt": np.ascontiguousarray(xt[:, :, sl]),
            "wloc": wloc,
            "wwin": wwin_g,
            "wgl": wgl,
            "whl": whl,
            "csm": csm,
            "ccol": ccol,
            "bg": bg,
        })
    return in_maps


def kernel(**inputs) -> np.ndarray:
    from concourse.bass_utils import run_bass_kernel_spmd

    if "nc" not in _CACHE:
        _CACHE["nc"] = _build_nc()
    nc = _CACHE["nc"]
    in_maps = _host_prep(inputs)
    res = run_bass_kernel_spmd(nc, in_maps, core_ids=list(range(N_CORES)))
    out = np.concatenate([res.results[c]["out"] for c in range(N_CORES)], axis=0)
    return out.astype(np.float32)
